# revision 1
# baseline (speedup 1.0000x reference)
"""Trainium2 Bass kernel for a single attention layer (Baichuan-style W_pack
attention with rotary embeddings), sharded over 8 NeuronCores:
tensor-parallel over 4 head groups x data-parallel over 2 batches.

v2: all-bf16 datapath (precision sim: max-rel ~4e-3 vs 2e-2 budget),
SBUF-resident q/k/v (no DRAM scratch round trips), exact causal trim,
single shared triangle mask, act-engine PSUM drains, pre-tiled weights
for wide DMA lines.

Contract: kernel(**inputs) takes the FULL unsharded inputs and returns the
FULL output [2, 2048, 4096] float32. All sharding / gathering happens here.
"""

import math
import sys

import numpy as np

for _p in ("/opt/trn_rl_repo", "/root/.axon_site/_ro/trn_rl_repo"):
    if _p not in sys.path:
        sys.path.insert(0, _p)

HIDDEN = 4096
N_HEADS = 32
HEAD_DIM = 128
BASE = 10000.0
B = 2
S = 2048
HEADS_PER_CORE = 8          # 32 heads / 4 groups
HG = 1024                   # head-group width = 8 heads * 128
NEG_BIG = -1.0e9

# RoPE partner permutation: quadrant q holds [lo_d 16q..16q+15, hi_d 64+16q..]
# so the rotate-half partner of new-row i is i+-16 inside its 32-row quadrant,
# reachable by DVE stream_shuffle.
PERM = np.zeros(128, dtype=np.int64)
for _q in range(4):
    PERM[32 * _q: 32 * _q + 16] = np.arange(16 * _q, 16 * _q + 16)
    PERM[32 * _q + 16: 32 * _q + 32] = 64 + np.arange(16 * _q, 16 * _q + 16)
SHUF_MASK = [(i + 16) % 32 for i in range(32)]
# sign of the sin term per (new) row: -1 where original d < 64
SIGN = np.where(PERM < 64, -1.0, 1.0).astype(np.float32)


def _rope_tables(max_pos):
    inv_freq = 1.0 / (BASE ** (np.arange(0, HEAD_DIM, 2, dtype=np.float32) / HEAD_DIM))
    t = np.arange(max_pos, dtype=np.float32)
    freqs = np.outer(t, inv_freq)                      # [P, 64]
    emb = np.concatenate((freqs, freqs), axis=-1)      # [P, 128]
    return np.cos(emb).astype(np.float32), np.sin(emb).astype(np.float32)


def _build_program(mask_mode):
    """mask_mode: 'causal' (block-skip + shared triangle mask),
    'none' (dense, no mask), 'full' (dense, stream mask tiles)."""
    import concourse.bacc as bacc
    import concourse.mybir as mybir
    import concourse.tile as tile
    from contextlib import ExitStack

    F32 = mybir.dt.float32
    BF16 = mybir.dt.bfloat16
    ALU = mybir.AluOpType
    ACTF = mybir.ActivationFunctionType

    nc = bacc.Bacc("TRN2", target_bir_lowering=False, debug=False)

    # pre-tiled inputs (host side does all layout work)
    x_t = nc.declare_dram_parameter("x_t", [128, 32, S], BF16, isOutput=False)
    wqk_t = nc.declare_dram_parameter("wqk_t", [16, 128, 32, 128], BF16,
                                      isOutput=False)
    wv_t = nc.declare_dram_parameter("wv_t", [2, 32, 128, 512], BF16,
                                     isOutput=False)
    wo_t = nc.declare_dram_parameter("wo_t", [128, 8, HIDDEN], BF16,
                                     isOutput=False)
    cosT = nc.declare_dram_parameter("cosT", [128, S], BF16, isOutput=False)
    sinT = nc.declare_dram_parameter("sinT", [128, S], BF16, isOutput=False)
    if mask_mode == "causal":
        tri = nc.declare_dram_parameter("tri", [128, 128], F32, isOutput=False)
    elif mask_mode == "full":
        maskT = nc.declare_dram_parameter("maskT", [S, S], F32, isOutput=False)
    out_p = nc.declare_dram_parameter("out_p", [S, HIDDEN], BF16, isOutput=True)

    at_s = nc.dram_tensor("at_scratch", [HG, S], BF16)

    inv_sqrt_d = 1.0 / math.sqrt(HEAD_DIM)

    with tile.TileContext(nc, pool_alloc_mode="queue") as tc, ExitStack() as top:
        const_pool = top.enter_context(tc.tile_pool(name="consts", bufs=1))
        ones_f32 = const_pool.tile([128, 1], F32)
        nc.vector.memset(ones_f32, 1.0)
        ones_col = const_pool.tile([128, 1], BF16)
        nc.vector.tensor_copy(ones_col, ones_f32)
        ones_row_f32 = const_pool.tile([1, 128], F32)
        nc.vector.memset(ones_row_f32, 1.0)
        ones_row = const_pool.tile([1, 128], BF16)
        nc.vector.tensor_copy(ones_row, ones_row_f32)
        if mask_mode == "causal":
            tri_sb = const_pool.tile([128, 128], F32)
            nc.sync.dma_start(out=tri_sb, in_=tri.ap())

        # SBUF-resident q/k/v, one tile per seq half to keep deps precise
        res_pool = top.enter_context(tc.tile_pool(name="resident", bufs=1))
        qT_sb = [res_pool.tile([128, HEADS_PER_CORE, 1024], BF16,
                               name=f"qT{h}") for h in range(2)]
        kT_sb = [res_pool.tile([128, HEADS_PER_CORE, 1024], BF16,
                               name=f"kT{h}") for h in range(2)]
        v_sb = [res_pool.tile([128, 8, HG], BF16, name=f"v{h}")
                for h in range(2)]

        # ---------------- Phase A: QKV projection (+RoPE on q,k) -------------
        def emit_proj_half(hs):
            s0 = hs * 1024
            # h-chunk sweep order: B-block (16..31) first (its DMA is issued
            # first), A-block (0..15) last.
            H_ORDER = list(range(16, 32)) + list(range(16))
            with ExitStack() as pha:
                xpoolA = pha.enter_context(tc.tile_pool(name="xhalfA", bufs=1))
                xpoolB = pha.enter_context(tc.tile_pool(name="xhalfB", bufs=1))
                xtA = xpoolA.tile([128, 16, 1024], BF16, name=f"xtA{hs}")
                xtB = xpoolB.tile([128, 16, 1024], BF16, name=f"xtB{hs}")
                xin = x_t.ap()[:, :, s0:s0 + 1024]
                for c in range(16, 32):
                    nc.sync.dma_start(out=xtB[:, c - 16, :], in_=xin[:, c, :])
                for c in range(16):
                    nc.sync.dma_start(out=xtA[:, c, :], in_=xin[:, c, :])

                def xt_slice(c, sl):
                    return xtB[:, c - 16, sl] if c >= 16 else xtA[:, c, sl]

                cspool = pha.enter_context(tc.tile_pool(name="cossin", bufs=1))
                cos_sb = cspool.tile([128, 1024], BF16)
                nc.sync.dma_start(out=cos_sb, in_=cosT.ap()[:, s0:s0 + 1024])
                sin_sb = cspool.tile([128, 1024], BF16)
                nc.sync.dma_start(out=sin_sb, in_=sinT.ap()[:, s0:s0 + 1024])

                # --- q,k projection, weight-stationary, out = projT [o, s] ---
                with ExitStack() as qk:
                    wpool = qk.enter_context(tc.tile_pool(name="wqk", bufs=2))
                    pqk = qk.enter_context(
                        tc.tile_pool(name="pqk", bufs=2, space="PSUM"))
                    rpool = qk.enter_context(tc.tile_pool(name="rope", bufs=2))
                    for oc in range(16):         # o chunks of 128 (head tiles)
                        w_oc = wpool.tile([128, 32, 128], BF16, tag="w_oc")
                        if oc == 0:
                            with tc.high_priority():
                                nc.sync.dma_start(out=w_oc, in_=wqk_t.ap()[oc])
                        else:
                            nc.sync.dma_start(out=w_oc, in_=wqk_t.ap()[oc])
                        pk = pqk.tile([128, 2, 512], F32, tag="pk")
                        for hi, h in enumerate(H_ORDER):
                            for sc in range(2):
                                nc.tensor.matmul(
                                    pk[:, sc, :], w_oc[:, h, :],
                                    xt_slice(h, slice(sc * 512, (sc + 1) * 512)),
                                    start=(hi == 0), stop=(hi == 31))
                        # RoPE: q' = q*cos + shuffle16(q)*sin_signed, -> bf16
                        dst = qT_sb[hs] if oc < 8 else kT_sb[hs]
                        hh = oc % 8
                        for sc in range(2):
                            pks = pk[:, sc, :]
                            cs = cos_sb[:, sc * 512:(sc + 1) * 512]
                            sn = sin_sb[:, sc * 512:(sc + 1) * 512]
                            qrot = rpool.tile([128, 512], F32, tag="qrot")
                            nc.vector.stream_shuffle(qrot, pks, SHUF_MASK)
                            t1 = rpool.tile([128, 512], F32, tag="t1")
                            nc.vector.tensor_tensor(t1, pks, cs, ALU.mult)
                            t2 = rpool.tile([128, 512], F32, tag="t2")
                            nc.gpsimd.tensor_tensor(t2, qrot, sn, ALU.mult)
                            nc.vector.tensor_tensor(
                                dst[:, hh, sc * 512:(sc + 1) * 512],
                                t1, t2, ALU.add)

                # --- v projection, x-stationary, out = v [s, o] --------------
                with ExitStack() as vv:
                    wvp = vv.enter_context(tc.tile_pool(name="wvt", bufs=4))
                    pv = vv.enter_context(
                        tc.tile_pool(name="pv", bufs=8, space="PSUM"))
                    for ov in range(2):          # v-dim chunks of 512
                        vb = [pv.tile([128, 512], F32, tag="vb", name=f"vb{i}")
                              for i in range(8)]
                        for hi, h in enumerate(H_ORDER):
                            wv_tile = wvp.tile([128, 512], BF16, tag="wv_tile")
                            nc.sync.dma_start(out=wv_tile, in_=wv_t.ap()[ov, h])
                            for sc in range(8):
                                nc.tensor.matmul(
                                    vb[sc],
                                    xt_slice(h, slice(sc * 128, (sc + 1) * 128)),
                                    wv_tile,
                                    start=(hi == 0), stop=(hi == 31))
                        for sc in range(8):
                            dst = v_sb[hs][:, sc, ov * 512:(ov + 1) * 512]
                            if sc % 2 == 0:
                                nc.scalar.activation(dst, vb[sc], ACTF.Copy)
                            else:
                                nc.vector.tensor_copy(dst, vb[sc])

        # ---------------- Phase B: attention, scores kept as S^T [k, q] ------
        # the four es tiles of each kb-quad are tree-summed on DVE so the PE
        # only runs one [1,512] den matmul per quad (gpsimd zeroes the unset
        # low-q slivers of diagonal-quad es tiles first).
        def emit_attn(qcs, mp_ctx):
            with ExitStack() as phb:
                qp_es = phb.enter_context(tc.tile_pool(name="es", bufs=6))
                esump = phb.enter_context(tc.tile_pool(name="esum", bufs=3))
                smallp = phb.enter_context(tc.tile_pool(name="small", bufs=3))
                ps = phb.enter_context(
                    tc.tile_pool(name="ps", bufs=3, space="PSUM"))
                pav = phb.enter_context(
                    tc.tile_pool(name="pav", bufs=2, space="PSUM"))
                pden = phb.enter_context(
                    tc.tile_pool(name="pden", bufs=2, space="PSUM"))
                pbc = phb.enter_context(
                    tc.tile_pool(name="pbc", bufs=1, space="PSUM"))
                mp = None
                if mask_mode == "full":
                    mp = phb.enter_context(tc.tile_pool(name="msk", bufs=3))

                # finalize is deferred by one head so the PE never stalls on
                # the recip chain: head h's softmax division is emitted after
                # head h+1's score/av blocks.
                def finalize(av, den, hh, qc):
                    recip_f = smallp.tile([1, 512], F32, tag="recip_f")
                    nc.vector.reciprocal_approx_fast(recip_f, den)
                    recip = smallp.tile([1, 512], BF16, tag="recip")
                    nc.vector.tensor_copy(recip, recip_f)
                    bc = pbc.tile([128, 512], F32, tag="bc")
                    nc.tensor.matmul(bc, ones_row, recip, start=True, stop=True)
                    bc_sb = smallp.tile([128, 512], F32, tag="bc_sb")
                    nc.scalar.activation(bc_sb, bc, ACTF.Copy)
                    at_t = smallp.tile([128, 512], BF16, tag="at_t")
                    nc.vector.tensor_tensor(at_t, av, bc_sb, ALU.mult)
                    nc.sync.dma_start(
                        out=at_s.ap()[hh * 128:(hh + 1) * 128,
                                      qc * 512:(qc + 1) * 512],
                        in_=at_t)

                pending = None
                for qc in qcs:
                    for hh in range(HEADS_PER_CORE):
                        nblk = 4 * qc + 4 if mask_mode == "causal" else 16
                        nquad = nblk // 4
                        av = pav.tile([128, 512], F32, tag="av")
                        den = pden.tile([1, 512], F32, tag="den")
                        for quad in range(nquad):
                            ess = []
                            for j in range(4):
                                kb = 4 * quad + j
                                vd = kb - 4 * qc   # diagonal block index
                                q_lo = (128 * vd
                                        if (mask_mode == "causal" and vd > 0)
                                        else 0)
                                qs = slice(q_lo, 512)
                                khalf, kloc = kb // 8, kb % 8
                                sps = ps.tile([128, 512], F32, tag="sps")
                                nc.tensor.matmul(
                                    sps[:, qs],
                                    kT_sb[khalf][:, hh,
                                                 kloc * 128:(kloc + 1) * 128],
                                    qT_sb[qc // 2][:, hh,
                                                   (qc % 2) * 512 + q_lo:
                                                   (qc % 2) * 512 + 512],
                                    start=True, stop=True)
                                if mask_mode == "causal" and 0 <= vd:
                                    mq = slice(128 * vd, 128 * vd + 128)
                                    nc.vector.tensor_tensor(
                                        sps[:, mq], sps[:, mq], tri_sb, ALU.add)
                                elif mask_mode == "full":
                                    mt = mp.tile([128, 512], F32, tag="mt")
                                    nc.sync.dma_start(
                                        out=mt,
                                        in_=maskT.ap()[kb * 128:(kb + 1) * 128,
                                                       qc * 512:(qc + 1) * 512])
                                    nc.vector.tensor_tensor(sps, sps, mt,
                                                            ALU.add)
                                es = qp_es.tile([128, 512], BF16, tag="es")
                                if q_lo > 0:
                                    nc.gpsimd.memset(es[:, 0:q_lo], 0.0)
                                nc.scalar.activation(es[:, qs], sps[:, qs],
                                                     ACTF.Exp, scale=inv_sqrt_d)
                                ess.append(es)
                                nc.tensor.matmul(
                                    av[:, qs],
                                    v_sb[khalf][:, kloc,
                                                hh * 128:(hh + 1) * 128],
                                    es[:, qs],
                                    start=(kb == 0), stop=(kb == nblk - 1))
                            e01 = esump.tile([128, 512], BF16, tag="e01")
                            nc.vector.tensor_tensor(e01, ess[0], ess[1],
                                                    ALU.add)
                            e23 = esump.tile([128, 512], BF16, tag="e23")
                            nc.vector.tensor_tensor(e23, ess[2], ess[3],
                                                    ALU.add)
                            nc.vector.tensor_tensor(e01, e01, e23, ALU.add)
                            nc.tensor.matmul(
                                den, ones_col, e01,
                                start=(quad == 0), stop=(quad == nquad - 1))
                        if pending is not None:
                            finalize(*pending)
                        pending = (av, den, hh, qc)
                if pending is not None:
                    finalize(*pending)

        emit_proj_half(0)
        if mask_mode == "causal":
            emit_attn((0, 1), None)    # q<1024 only needs k<1024 (half 0)
            emit_proj_half(1)
            wop = top.enter_context(tc.tile_pool(name="wo", bufs=1))
            wo_sb = wop.tile([128, 8, HIDDEN], BF16)
            nc.sync.dma_start(out=wo_sb, in_=wo_t.ap())   # overlaps B23
            emit_attn((2, 3), None)
        else:
            emit_proj_half(1)
            wop = top.enter_context(tc.tile_pool(name="wo", bufs=1))
            wo_sb = wop.tile([128, 8, HIDDEN], BF16)
            nc.sync.dma_start(out=wo_sb, in_=wo_t.ap())
            emit_attn((0, 1, 2, 3), None)

        # ---------------- Phase C: output projection -------------------------
        # oc-outer accumulation: each PSUM bank finishes early in the st tile's
        # stream, so its drain (act copy) overlaps the remaining matmuls.
        with ExitStack() as phc:
            atp = phc.enter_context(tc.tile_pool(name="atl", bufs=2))
            pop = phc.enter_context(tc.tile_pool(name="pop", bufs=8,
                                                 space="PSUM"))
            otp = phc.enter_context(tc.tile_pool(name="ot", bufs=2))
            for stg in range(4):                 # s groups of 512
                at_g = atp.tile([128, 8, 512], BF16, tag="at_g")
                nc.sync.dma_start(
                    out=at_g,
                    in_=at_s.ap()[:, stg * 512:(stg + 1) * 512].rearrange(
                        "(hc p) s -> p hc s", p=128))
                for st_l in range(4):            # s tiles of 128
                    sl = slice(st_l * 128, (st_l + 1) * 128)
                    st = stg * 4 + st_l
                    ot = otp.tile([128, 8, 512], BF16, tag="ot")
                    for o8 in range(8):          # output chunks of 512
                        po = pop.tile([128, 512], F32, tag="po")
                        for hc in range(8):
                            nc.tensor.matmul(
                                po,
                                at_g[:, hc, sl],
                                wo_sb[:, hc, o8 * 512:(o8 + 1) * 512],
                                start=(hc == 0), stop=(hc == 7))
                        nc.scalar.activation(ot[:, o8, :], po, ACTF.Copy)
                    nc.sync.dma_start(
                        out=out_p.ap()[st * 128:(st + 1) * 128, :],
                        in_=ot.rearrange("p a b -> p (a b)"))

    nc.compile()
    return nc


_PROGRAM_CACHE = {}


def _get_program(mask_mode):
    if mask_mode not in _PROGRAM_CACHE:
        _PROGRAM_CACHE[mask_mode] = _build_program(mask_mode)
    return _PROGRAM_CACHE[mask_mode]


def _classify_mask(attention_mask):
    m = np.asarray(attention_mask)
    if not np.any(m):
        return "none"
    neg = np.float32(np.finfo(np.float32).min)
    causal = np.triu(np.full((S, S), neg, dtype=np.float32), k=1)
    for b in range(m.shape[0]):
        if not np.array_equal(m[b, 0], causal):
            return "full"
    return "causal"


def _prep_core_inputs(hidden_states, attention_mask, position_ids, W_pack, W_o,
                      mask_mode):
    from ml_dtypes import bfloat16

    hidden_states = np.asarray(hidden_states, dtype=np.float32)
    W_pack = np.asarray(W_pack, dtype=np.float32)
    W_o = np.asarray(W_o, dtype=np.float32)
    pos = np.asarray(position_ids).astype(np.int64)

    cos_t, sin_t = _rope_tables(int(pos.max()) + 1)
    # per-batch gathered + transposed + row-permuted (+ sign folded into sin)
    cosT_b, sinT_b = [], []
    for b in range(B):
        c = cos_t[pos[b]][:, PERM].T
        s = (sin_t[pos[b]][:, PERM] * SIGN[None, :]).T
        cosT_b.append(np.ascontiguousarray(c.astype(bfloat16)))
        sinT_b.append(np.ascontiguousarray(s.astype(bfloat16)))

    # x_t[p, c, s] = hidden[b, s, c*128+p]
    x_b = [np.ascontiguousarray(
        hidden_states[b].T.reshape(32, 128, S).transpose(1, 0, 2)
        .astype(bfloat16)) for b in range(B)]

    tri_m = None
    maskT_b = None
    if mask_mode == "causal":
        kk = np.arange(128)[:, None]
        qq = np.arange(128)[None, :]
        tri_m = np.where(kk <= qq, 0.0, NEG_BIG).astype(np.float32)
    elif mask_mode == "full":
        m = np.asarray(attention_mask, dtype=np.float32)
        maskT_b = [np.ascontiguousarray(m[b, 0].T) for b in range(B)]

    in_maps = []
    for cidx in range(8):
        b, g = cidx // 4, cidx % 4
        # per-head d-permuted q/k weight rows, head-major columns in wqk
        qrows = np.concatenate(
            [g * HG + hh * 128 + PERM for hh in range(HEADS_PER_CORE)])
        krows = HIDDEN + qrows
        vrows = 2 * HIDDEN + g * HG + np.arange(HG)
        wqk = np.concatenate([W_pack[qrows], W_pack[krows]], axis=0)  # [2048,4096]
        # wqk_t[oc, p, c, o] = wqk[oc*128+o, c*128+p]
        wqk_t = np.ascontiguousarray(
            wqk.reshape(16, 128, 32, 128).transpose(0, 3, 2, 1)
            .astype(bfloat16))
        wv = W_pack[vrows]                                            # [1024,4096]
        # wv_t[ov, c, p, o] = wv[ov*512+o, c*128+p]
        wv_t = np.ascontiguousarray(
            wv.reshape(2, 512, 32, 128).transpose(0, 2, 3, 1).astype(bfloat16))
        # wo_t[p, hc, o] = W_o[o, g*HG + hc*128 + p]
        wo_t = np.ascontiguousarray(
            W_o[:, g * HG:(g + 1) * HG].reshape(HIDDEN, 8, 128)
            .transpose(2, 1, 0).astype(bfloat16))
        im = {"x_t": x_b[b], "wqk_t": wqk_t, "wv_t": wv_t, "wo_t": wo_t,
              "cosT": cosT_b[b], "sinT": sinT_b[b]}
        if mask_mode == "causal":
            im["tri"] = tri_m
        elif mask_mode == "full":
            im["maskT"] = maskT_b[b]
        in_maps.append(im)
    return in_maps


def _run(hidden_states, attention_mask, position_ids, W_pack, W_o,
         trace=False, trace_kwargs=None):
    from concourse.bass_utils import run_bass_kernel_spmd

    mask_mode = _classify_mask(attention_mask)
    nc = _get_program(mask_mode)
    in_maps = _prep_core_inputs(hidden_states, attention_mask, position_ids,
                                W_pack, W_o, mask_mode)
    try:
        res = run_bass_kernel_spmd(nc, in_maps, list(range(8)), trace=trace,
                                   **(trace_kwargs or {}))
    except Exception:
        # transient NRT_EXEC_UNIT_UNRECOVERABLE wedges recover on retry
        import time as _time
        _time.sleep(15)
        res = run_bass_kernel_spmd(nc, in_maps, list(range(8)), trace=trace,
                                   **(trace_kwargs or {}))
    out = np.zeros((B, S, HIDDEN), dtype=np.float32)
    for c in range(8):
        out[c // 4] += np.asarray(res.results[c]["out_p"], dtype=np.float32)
    return out, res


def kernel(hidden_states, attention_mask, position_ids, W_pack, W_o):
    out, _ = _run(hidden_states, attention_mask, position_ids, W_pack, W_o)
    return out

